# revision 4
# baseline (speedup 1.0000x reference)
"""Strassen(fp16) + fp8-DoubleRow hybrid GEMM for nn_LogitsProjector.

Per core: C[2048, 3125] = A[2048, 32000] @ B[3125, 32000].T (no padding;
8*3125 = 25000). Strassen still splits the padded 3200 into 1600-halves,
but matmuls that feed only the ragged last n-block (width 725) are
trimmed, as are the fp8 tail blocks and the output.
 - Last KO8*128 of K in fp8e4m3 DoubleRow (~2x rate), accumulated per
   output block and added into the C accumulators.
 - First KO16*128 of K in fp16 via one Strassen level (M, N, K all
   halved; 7 products instead of 8 -> 12.5% less tensor-engine work).
   Operand combos (A11+A22 etc.) are precomputed host-side in fp16.
 - C-quadrant combination runs on the vector engine into SBUF
   accumulators, overlapped with the next product's matmuls.
"""

import numpy as np

P = 128
N_TOK = 2048
K = 32000
SV = 25000
N_CORES = 8
NPC = 3125             # real output cols per core
NPC_PAD = 3200         # strassen halves need the padded width
MB = 512
NB = 800
KO = K // P            # 250
KO8 = 64               # fp8 k-tiles (even)
KO16 = KO - KO8        # 186 fp16 k-tiles; half = 93
CKS = 3                # strassen k-tiles per DMA chunk (divides KO16//2)
CK8 = 16               # fp8 k-tiles per chunk

# product -> (A-combo spec, B-combo spec) with blocks 1=11,2=12,3=21,4=22
# combos: (x, y, sign) meaning block_x + sign*block_y  (y=0 -> just x)
A_COMBOS = [(1, 4, 1), (3, 4, 1), (1, 0, 1), (4, 0, 1), (1, 2, 1),
            (3, 1, -1), (2, 4, -1)]
B_COMBOS = [(1, 4, 1), (1, 0, 1), (3, 4, -1), (2, 1, -1), (4, 0, 1),
            (1, 3, 1), (2, 4, 1)]
# product -> list of (quadrant, sign); quadrants 0=C11 1=C12 2=C21 3=C22
SIGNS = [[(0, 1), (3, 1)], [(2, 1), (3, -1)], [(1, 1), (3, 1)],
         [(0, 1), (2, 1)], [(0, -1), (1, 1)], [(3, 1)], [(0, 1)]]
# products whose outputs land only in C12/C22 (vocab cols >= 1600):
# at j==1 their last 75 columns are padding and can be trimmed.
RAGGED_PRODS = {p for p, sg in enumerate(SIGNS)
                if all(q in (1, 3) for q, _ in sg)}  # {2, 5}

_cache = {}


def _build(ko16=KO16, ko8=KO8, cks=CKS, ck8=CK8):
    import concourse.bacc as bacc
    import concourse.mybir as mybir
    import concourse.tile as tile

    f16 = mybir.dt.float16
    f8 = mybir.dt.float8e4
    f32 = mybir.dt.float32
    DR = mybir.MatmulPerfMode.DoubleRow
    ADD = mybir.AluOpType.add
    SUB = mybir.AluOpType.subtract

    k2 = ko16 // 2
    assert k2 % cks == 0 and ko8 % ck8 == 0 and ck8 % 2 == 0

    nc = bacc.Bacc(None, target_bir_lowering=False, debug=False)
    am = nc.dram_tensor("am", (P, 7, 2, k2, MB), f16, kind="ExternalInput")
    bm = nc.dram_tensor("bm", (P, 7, 2, k2, NB), f16, kind="ExternalInput")
    kxm8 = nc.dram_tensor("kxm8", (P, 4, ko8, MB), f8, kind="ExternalInput")
    kxn8 = nc.dram_tensor("kxn8", (P, 4, ko8, NB), f8, kind="ExternalInput")
    out = nc.dram_tensor("out", (P, N_TOK // P, NPC), f32, kind="ExternalOutput")

    with tile.TileContext(nc) as tc:
        with tc.tile_pool(name="apool", bufs=4) as apool, \
             tc.tile_pool(name="bpool", bufs=4) as bpool, \
             tc.tile_pool(name="a8pool", bufs=3) as a8pool, \
             tc.tile_pool(name="b8pool", bufs=3) as b8pool, \
             tc.tile_pool(name="cpool", bufs=1) as cpool, \
             tc.tile_pool(name="pspool", bufs=1, space="PSUM") as pspool:
            for i in range(2):
                for j in range(2):
                    c = [[cpool.tile([P, NB], f32, name=f"c{q}_{ms}")
                          for ms in range(4)] for q in range(4)]
                    touched = [False] * 4
                    for prod in range(7):
                        w = 725 if (j == 1 and prod in RAGGED_PRODS) else NB
                        ps = [pspool.tile([P, NB], f32, name=f"ps{s}")
                              for s in range(4)]
                        for kc in range(k2 // cks):
                            at = apool.tile([P, cks, MB], f16, name="a")
                            bt = bpool.tile([P, cks, NB], f16, name="b")
                            nc.sync.dma_start(
                                at[:], am[:, prod, i, kc * cks:(kc + 1) * cks, :])
                            nc.sync.dma_start(
                                bt[:], bm[:, prod, j, kc * cks:(kc + 1) * cks, :])
                            for ki in range(cks):
                                kg = kc * cks + ki
                                st, sp = kg == 0, kg == k2 - 1
                                for ms in range(4):
                                    lhsT = at[:, ki, ms * 128:(ms + 1) * 128]
                                    nc.tensor.matmul(ps[ms][:, 0:512], lhsT,
                                                     bt[:, ki, 0:512],
                                                     start=st, stop=sp)
                                    nc.tensor.matmul(ps[ms][:, 512:w], lhsT,
                                                     bt[:, ki, 512:w],
                                                     start=st, stop=sp)
                        for (q, sgn) in SIGNS[prod]:
                            qw = 725 if (j == 1 and q in (1, 3)) else NB
                            for ms in range(4):
                                if not touched[q]:
                                    nc.vector.tensor_copy(
                                        c[q][ms][:, 0:qw], ps[ms][:, 0:qw])
                                else:
                                    nc.vector.tensor_tensor(
                                        c[q][ms][:, 0:qw], c[q][ms][:, 0:qw],
                                        ps[ms][:, 0:qw],
                                        ADD if sgn > 0 else SUB)
                            touched[q] = True
                    # fp8 DoubleRow tail per quadrant block
                    for q in range(4):
                        mb = (q // 2) * 2 + i
                        nb = (q % 2) * 2 + j
                        bw = 725 if nb == 3 else NB
                        ps = [pspool.tile([P, NB], f32, name=f"ps{s}")
                              for s in range(4)]
                        for kc in range(ko8 // ck8):
                            at8 = a8pool.tile([P, ck8, MB], f8, name="a8")
                            bt8 = b8pool.tile([P, ck8, NB], f8, name="b8")
                            nc.sync.dma_start(
                                at8[:], kxm8[:, mb, kc * ck8:(kc + 1) * ck8, :])
                            nc.sync.dma_start(
                                bt8[:], kxn8[:, nb, kc * ck8:(kc + 1) * ck8, :])
                            for t in range(ck8 // 2):
                                st = kc == 0 and t == 0
                                sp = kc == ko8 // ck8 - 1 and t == ck8 // 2 - 1
                                for ms in range(4):
                                    lhsT = at8[:, 2 * t:2 * t + 2,
                                               ms * 128:(ms + 1) * 128]
                                    nc.tensor.matmul(
                                        ps[ms][:, 0:512], lhsT,
                                        bt8[:, 2 * t:2 * t + 2, 0:512],
                                        start=st, stop=sp, perf_mode=DR)
                                    nc.tensor.matmul(
                                        ps[ms][:, 512:bw], lhsT,
                                        bt8[:, 2 * t:2 * t + 2, 512:bw],
                                        start=st, stop=sp, perf_mode=DR)
                        for ms in range(4):
                            nc.vector.tensor_tensor(
                                c[q][ms][:, 0:bw], c[q][ms][:, 0:bw],
                                ps[ms][:, 0:bw], ADD)
                            nc.sync.dma_start(
                                out[:, mb * 4 + ms, nb * NB:nb * NB + bw],
                                c[q][ms][:, 0:bw])
    nc.compile()
    return nc


def _get_nc():
    if "nc" not in _cache:
        _cache["nc"] = _build()
    return _cache["nc"]


def _combo(blocks, spec):
    x, y, sgn = spec
    if y == 0:
        return blocks[x]
    return (blocks[x] + np.float16(sgn) * blocks[y]).astype(np.float16)


def _prep_strassen(x16, half_rows, k2p, combos):
    # x16: [2*half_rows, 2*k2p] fp16. blocks 1=11 2=12 3=21 4=22
    blocks = {1: x16[:half_rows, :k2p], 2: x16[:half_rows, k2p:],
              3: x16[half_rows:, :k2p], 4: x16[half_rows:, k2p:]}
    bs = half_rows // 2  # rows per sub-block (i index)
    k2 = k2p // P
    panels = []
    for spec in combos:
        cb = _combo(blocks, spec)  # [half_rows, k2p] fp16
        panels.append(cb.reshape(2, bs, k2, P).transpose(3, 0, 2, 1))
    return np.ascontiguousarray(np.stack(panels, axis=1))  # (P,7,2,k2,bs)


def _prep8(x, blks, bs, ko16, ko8):
    import ml_dtypes
    t = x[:, ko16 * P:(ko16 + ko8) * P].astype(ml_dtypes.float8_e4m3)
    return np.ascontiguousarray(
        t.reshape(blks, bs, ko8, P).transpose(3, 0, 2, 1))


def kernel(teacher_logits: np.ndarray, projection: np.ndarray) -> np.ndarray:
    from concourse.bass_utils import run_bass_kernel_spmd

    nc = _get_nc()

    teacher = np.asarray(teacher_logits, dtype=np.float32)
    proj = np.asarray(projection, dtype=np.float32)
    proj_pad = np.zeros((N_CORES * NPC_PAD, K), dtype=np.float32)
    for c in range(N_CORES):
        proj_pad[c * NPC_PAD:c * NPC_PAD + NPC] = proj[c * NPC:(c + 1) * NPC]

    k16p = KO16 * P
    a16 = teacher[:, :k16p].astype(np.float16)
    am_np = _prep_strassen(a16, N_TOK // 2, k16p // 2, A_COMBOS)
    kxm8_np = _prep8(teacher, 4, MB, KO16, KO8)

    in_maps = []
    for c in range(N_CORES):
        shard = proj_pad[c * NPC_PAD:(c + 1) * NPC_PAD]
        b16 = shard[:, :k16p].astype(np.float16)
        in_maps.append({
            "am": am_np,
            "bm": _prep_strassen(b16, NPC_PAD // 2, k16p // 2, B_COMBOS),
            "kxm8": kxm8_np,
            "kxn8": _prep8(shard, 4, NB, KO16, KO8),
        })

    res = run_bass_kernel_spmd(nc, in_maps, core_ids=list(range(N_CORES)))
    _cache["last_res"] = res

    parts = []
    for c in range(N_CORES):
        o = res.results[c]["out"]
        parts.append(o.transpose(1, 0, 2).reshape(N_TOK, NPC))
    full = np.concatenate(parts, axis=1)
    assert full.shape == (N_TOK, SV)
    return np.ascontiguousarray(full.astype(np.float32))


# revision 5
# speedup vs baseline: 1.0574x; 1.0574x over previous
"""Strassen(fp16) + fp8-DoubleRow hybrid GEMM for nn_LogitsProjector.

Per core: C[2048, 3125] = A[2048, 32000] @ B[3125, 32000].T (no padding;
8*3125 = 25000). Strassen still splits the padded 3200 into 1600-halves,
but matmuls that feed only the ragged last n-block (width 725) are
trimmed, as are the fp8 tail blocks and the output.
 - Last KO8*128 of K in fp8e4m3 DoubleRow (~2x rate), accumulated per
   output block and added into the C accumulators.
 - First KO16*128 of K in fp16 via one Strassen level (M, N, K all
   halved; 7 products instead of 8 -> 12.5% less tensor-engine work).
   Operand combos (A11+A22 etc.) are precomputed host-side in fp16.
 - C-quadrant combination runs on the vector engine into SBUF
   accumulators, overlapped with the next product's matmuls.
"""

import numpy as np

P = 128
N_TOK = 2048
K = 32000
SV = 25000
N_CORES = 8
NPC = 3125             # real output cols per core
NPC_PAD = 3200         # strassen halves need the padded width
MB = 512
NB = 800
KO = K // P            # 250
KO8 = 64               # fp8 k-tiles (even)
KO16 = KO - KO8        # 186 fp16 k-tiles; half = 93
CKS = 3                # strassen k-tiles per DMA chunk (divides KO16//2)
CK8 = 16               # fp8 k-tiles per chunk

# product -> (A-combo spec, B-combo spec) with blocks 1=11,2=12,3=21,4=22
# combos: (x, y, sign) meaning block_x + sign*block_y  (y=0 -> just x)
A_COMBOS = [(1, 4, 1), (3, 4, 1), (1, 0, 1), (4, 0, 1), (1, 2, 1),
            (3, 1, -1), (2, 4, -1)]
B_COMBOS = [(1, 4, 1), (1, 0, 1), (3, 4, -1), (2, 1, -1), (4, 0, 1),
            (1, 3, 1), (2, 4, 1)]
# product -> list of (quadrant, sign); quadrants 0=C11 1=C12 2=C21 3=C22
SIGNS = [[(0, 1), (3, 1)], [(2, 1), (3, -1)], [(1, 1), (3, 1)],
         [(0, 1), (2, 1)], [(0, -1), (1, 1)], [(3, 1)], [(0, 1)]]
# products whose outputs land only in C12/C22 (vocab cols >= 1600):
# at j==1 their last 75 columns are padding and can be trimmed.
RAGGED_PRODS = {p for p, sg in enumerate(SIGNS)
                if all(q in (1, 3) for q, _ in sg)}  # {2, 5}

_cache = {}


def _build(ko16=KO16, ko8=KO8, cks=CKS, ck8=CK8):
    import concourse.bacc as bacc
    import concourse.mybir as mybir
    import concourse.tile as tile

    f16 = mybir.dt.float16
    f8 = mybir.dt.float8e4
    f32 = mybir.dt.float32
    DR = mybir.MatmulPerfMode.DoubleRow
    ADD = mybir.AluOpType.add
    SUB = mybir.AluOpType.subtract

    k2 = ko16 // 2
    assert k2 % cks == 0 and ko8 % ck8 == 0 and ck8 % 2 == 0

    nc = bacc.Bacc(None, target_bir_lowering=False, debug=False)
    am = nc.dram_tensor("am", (P, 7, 2, k2, MB), f16, kind="ExternalInput")
    bm = nc.dram_tensor("bm", (P, 7, 2, k2, NB), f16, kind="ExternalInput")
    kxm8 = nc.dram_tensor("kxm8", (P, 4, ko8, MB), f8, kind="ExternalInput")
    kxn8 = nc.dram_tensor("kxn8", (P, 4, ko8, NB), f8, kind="ExternalInput")
    out = nc.dram_tensor("out", (P, N_TOK // P, NPC), f32, kind="ExternalOutput")

    with tile.TileContext(nc) as tc:
        with tc.tile_pool(name="apool", bufs=4) as apool, \
             tc.tile_pool(name="bpool", bufs=4) as bpool, \
             tc.tile_pool(name="a8pool", bufs=3) as a8pool, \
             tc.tile_pool(name="b8pool", bufs=3) as b8pool, \
             tc.tile_pool(name="cpool", bufs=1) as cpool, \
             tc.tile_pool(name="pspool", bufs=1, space="PSUM") as pspool:
            for i in range(2):
                for j in range(2):
                    c = [[cpool.tile([P, NB], f32, name=f"c{q}_{ms}")
                          for ms in range(4)] for q in range(4)]
                    touched = [False] * 4
                    for prod in range(7):
                        w = 725 if (j == 1 and prod in RAGGED_PRODS) else NB
                        ps = [pspool.tile([P, NB], f32, name=f"ps{s}")
                              for s in range(4)]
                        for kc in range(k2 // cks):
                            at = apool.tile([P, cks, MB], f16, name="a")
                            bt = bpool.tile([P, cks, NB], f16, name="b")
                            nc.sync.dma_start(
                                at[:], am[:, prod, i, kc * cks:(kc + 1) * cks, :])
                            nc.sync.dma_start(
                                bt[:], bm[:, prod, j, kc * cks:(kc + 1) * cks, :])
                            for ki in range(cks):
                                kg = kc * cks + ki
                                st, sp = kg == 0, kg == k2 - 1
                                for ms in range(4):
                                    lhsT = at[:, ki, ms * 128:(ms + 1) * 128]
                                    nc.tensor.matmul(ps[ms][:, 0:512], lhsT,
                                                     bt[:, ki, 0:512],
                                                     start=st, stop=sp)
                                    nc.tensor.matmul(ps[ms][:, 512:w], lhsT,
                                                     bt[:, ki, 512:w],
                                                     start=st, stop=sp)
                        # ms-major so ps[ms] is released after its own
                        # 1-2 adds (next product's first matmul waits on
                        # ps[0] only)
                        for ms in range(4):
                            for (q, sgn) in SIGNS[prod]:
                                qw = 725 if (j == 1 and q in (1, 3)) else NB
                                if not touched[q]:
                                    nc.vector.tensor_copy(
                                        c[q][ms][:, 0:qw], ps[ms][:, 0:qw])
                                else:
                                    nc.vector.tensor_tensor(
                                        c[q][ms][:, 0:qw], c[q][ms][:, 0:qw],
                                        ps[ms][:, 0:qw],
                                        ADD if sgn > 0 else SUB)
                            if ms == 3:
                                for (q, _sgn) in SIGNS[prod]:
                                    touched[q] = True
                    # fp8 DoubleRow tail per quadrant block
                    for q in range(4):
                        mb = (q // 2) * 2 + i
                        nb = (q % 2) * 2 + j
                        bw = 725 if nb == 3 else NB
                        ps = [pspool.tile([P, NB], f32, name=f"ps{s}")
                              for s in range(4)]
                        for kc in range(ko8 // ck8):
                            at8 = a8pool.tile([P, ck8, MB], f8, name="a8")
                            bt8 = b8pool.tile([P, ck8, NB], f8, name="b8")
                            nc.sync.dma_start(
                                at8[:], kxm8[:, mb, kc * ck8:(kc + 1) * ck8, :])
                            nc.sync.dma_start(
                                bt8[:], kxn8[:, nb, kc * ck8:(kc + 1) * ck8, :])
                            for t in range(ck8 // 2):
                                st = kc == 0 and t == 0
                                sp = kc == ko8 // ck8 - 1 and t == ck8 // 2 - 1
                                for ms in range(4):
                                    lhsT = at8[:, 2 * t:2 * t + 2,
                                               ms * 128:(ms + 1) * 128]
                                    nc.tensor.matmul(
                                        ps[ms][:, 0:512], lhsT,
                                        bt8[:, 2 * t:2 * t + 2, 0:512],
                                        start=st, stop=sp, perf_mode=DR)
                                    nc.tensor.matmul(
                                        ps[ms][:, 512:bw], lhsT,
                                        bt8[:, 2 * t:2 * t + 2, 512:bw],
                                        start=st, stop=sp, perf_mode=DR)
                        for ms in range(4):
                            nc.vector.tensor_tensor(
                                c[q][ms][:, 0:bw], c[q][ms][:, 0:bw],
                                ps[ms][:, 0:bw], ADD)
                            nc.sync.dma_start(
                                out[:, mb * 4 + ms, nb * NB:nb * NB + bw],
                                c[q][ms][:, 0:bw])
    nc.compile()
    return nc


def _get_nc():
    if "nc" not in _cache:
        _cache["nc"] = _build()
    return _cache["nc"]


def _combo(blocks, spec):
    x, y, sgn = spec
    if y == 0:
        return blocks[x]
    return (blocks[x] + np.float16(sgn) * blocks[y]).astype(np.float16)


def _prep_strassen(x16, half_rows, k2p, combos):
    # x16: [2*half_rows, 2*k2p] fp16. blocks 1=11 2=12 3=21 4=22
    blocks = {1: x16[:half_rows, :k2p], 2: x16[:half_rows, k2p:],
              3: x16[half_rows:, :k2p], 4: x16[half_rows:, k2p:]}
    bs = half_rows // 2  # rows per sub-block (i index)
    k2 = k2p // P
    panels = []
    for spec in combos:
        cb = _combo(blocks, spec)  # [half_rows, k2p] fp16
        panels.append(cb.reshape(2, bs, k2, P).transpose(3, 0, 2, 1))
    return np.ascontiguousarray(np.stack(panels, axis=1))  # (P,7,2,k2,bs)


def _prep8(x, blks, bs, ko16, ko8):
    import ml_dtypes
    t = x[:, ko16 * P:(ko16 + ko8) * P].astype(ml_dtypes.float8_e4m3)
    return np.ascontiguousarray(
        t.reshape(blks, bs, ko8, P).transpose(3, 0, 2, 1))


def kernel(teacher_logits: np.ndarray, projection: np.ndarray) -> np.ndarray:
    from concourse.bass_utils import run_bass_kernel_spmd

    nc = _get_nc()

    teacher = np.asarray(teacher_logits, dtype=np.float32)
    proj = np.asarray(projection, dtype=np.float32)
    proj_pad = np.zeros((N_CORES * NPC_PAD, K), dtype=np.float32)
    for c in range(N_CORES):
        proj_pad[c * NPC_PAD:c * NPC_PAD + NPC] = proj[c * NPC:(c + 1) * NPC]

    k16p = KO16 * P
    a16 = teacher[:, :k16p].astype(np.float16)
    am_np = _prep_strassen(a16, N_TOK // 2, k16p // 2, A_COMBOS)
    kxm8_np = _prep8(teacher, 4, MB, KO16, KO8)

    in_maps = []
    for c in range(N_CORES):
        shard = proj_pad[c * NPC_PAD:(c + 1) * NPC_PAD]
        b16 = shard[:, :k16p].astype(np.float16)
        in_maps.append({
            "am": am_np,
            "bm": _prep_strassen(b16, NPC_PAD // 2, k16p // 2, B_COMBOS),
            "kxm8": kxm8_np,
            "kxn8": _prep8(shard, 4, NB, KO16, KO8),
        })

    res = run_bass_kernel_spmd(nc, in_maps, core_ids=list(range(N_CORES)))
    _cache["last_res"] = res

    parts = []
    for c in range(N_CORES):
        o = res.results[c]["out"]
        parts.append(o.transpose(1, 0, 2).reshape(N_TOK, NPC))
    full = np.concatenate(parts, axis=1)
    assert full.shape == (N_TOK, SV)
    return np.ascontiguousarray(full.astype(np.float32))
